# revision 25
# baseline (speedup 1.0000x reference)
"""EulerAttentionHead Trainium2 kernel (8 NeuronCores, SPMD).

Reference computation (B=4, S=4096, D=1024, H=128):
    Q = x @ Wq.T + bq ; K = x @ Wk.T + bk ; V = x @ Wv.T + bv
    theta_{q,k} = {Q,K} / (wavelengths + 1e-8) + phase_bias
    sim = cos(tq) @ cos(tk).T + sin(tq) @ sin(tk).T
    out = softmax(sim / sqrt(H)) @ V @ Wo.T + bo

Sharding: 8 cores = 4 batches x 2 query-halves. Each core handles one
batch's full key/value set (4096 keys) and 2048 queries. The host rolls
x so each core's query rows are rows 0:2048 of its input (softmax over
keys is permutation-invariant, so key order doesn't matter).

Host prep: x and the four weight matrices are cast to fp16 and
pre-transposed in numpy so every device-side matmul operand has its
contraction dim on SBUF partitions. bv is folded into bo on the host
(softmax weights sum to 1, so + bv commutes through the attention
average into the output projection: bo' = bo + Wo @ bv).

Per-core dataflow:
  phase A: per 512-row chunk, Q.T/K.T/V.T = W.T-stationary fp16 matmuls
    over x.T; theta built on DVE (per-partition scale/bias, fp32 magic-
    number round, Cody-Waite cascade + add_range_wrap into the ACT Sin
    domain); sin/cos emitted as fp8e4 planes of [128, 2, n] tiles
    (plane 0 = cos, plane 1 = sin) for DoubleRow score matmuls. V is
    re-transposed to natural [k, h] layout on the PE with a ones column
    appended.
  phase B: per 512-query chunk, S.T k-tiles [k,128 x q,512] computed as
    ONE fp8 DoubleRow matmul each (contracts cos&sin planes, 2x FLOP
    rate); 3 k-tiles pooled per 3-bank PSUM tile so one ACT pass
    computes exp(S/sqrt(H) - 1) -> E.T fp16 over 1536 cols (amortizes
    the 352-cycle ACT fixed cost). AV: lhsT = E.T, rhs = [V | ones], so
    the softmax denominator accumulates as PSUM column 128 for free.
    Raw [O | denom] is evicted to SBUF; reciprocals run on DVE.
  phase C: normalize O rows on DVE, PE-transpose O, project with Wo.T,
    bias added on DVE during eviction, store on alternating HWDGE
    queues.

PSUM per phase: A: proj + V transpose; B: 2x 3-bank S.T tiles + 2 banks
of packed O accumulators (start=True zeroes a whole 2KB bank, so the
packed accumulators carry exactly one start per bank); C: O transpose +
output tiles.
"""

import math

import numpy as np

import concourse.mybir as mybir
import concourse.tile as tile
from concourse import bacc

F32 = mybir.dt.float32
F16 = mybir.dt.float16
F8 = mybir.dt.float8e4
AF = mybir.ActivationFunctionType
DR = mybir.MatmulPerfMode.DoubleRow

B, S, D, H = 4, 4096, 1024, 128
SQ = S // 2  # queries per core
N_CORES = 8

TWO_PI = 2.0 * math.pi
INV_TWO_PI = 1.0 / TWO_PI
MAGIC = 12582912.0  # 1.5 * 2**23: fp32 (u + M) - M == round(u)
INV_SQRT_H = 1.0 / math.sqrt(H)


def _cody_waite_consts():
    # Split 2*pi into c1 + c2 + c3, c1/c2 with zeroed low mantissa bits so
    # theta - k*c1 - k*c2 - k*c3 cancels exactly for small integer k.
    def chop(v):
        f = np.float32(v)
        i = f.view(np.uint32) & np.uint32(0xFFFFF000)
        return float(i.view(np.float32))

    c1 = chop(TWO_PI)
    c2 = chop(TWO_PI - c1)
    c3 = float(np.float32(TWO_PI - c1 - c2))
    return c1, c2, c3


C1, C2, C3 = _cody_waite_consts()

_CACHED = None


def _build():
    nc = bacc.Bacc("TRN2", target_bir_lowering=False, debug=False,
                   num_devices=N_CORES)

    xT = nc.dram_tensor("xT", (D, S), F16, kind="ExternalInput")
    WqTd = nc.dram_tensor("WqT", (D, H), F16, kind="ExternalInput")
    WkTd = nc.dram_tensor("WkT", (D, H), F16, kind="ExternalInput")
    WvTd = nc.dram_tensor("WvT", (D, H), F16, kind="ExternalInput")
    WoTd = nc.dram_tensor("WoT", (H, D), F16, kind="ExternalInput")
    vecs = nc.dram_tensor("vecs", (H, 4), F32, kind="ExternalInput")
    # y is stored fp16 (half the store bandwidth); host upcasts + adds bo.
    y = nc.dram_tensor("y", (SQ, D), F16, kind="ExternalOutput")

    with tile.TileContext(nc) as tc, \
            tc.tile_pool(name="const", bufs=1) as const, \
            tc.tile_pool(name="big", bufs=1) as big, \
            tc.tile_pool(name="xa", bufs=2) as xa_pool, \
            tc.tile_pool(name="tmp", bufs=3) as tmp:

        # ---- input DMAs, spread across engine trigger queues so the
        # critical-path loads (WkT, xt0) are triggered by sequencers that
        # go idle earliest and don't queue behind 8 MB of x descriptors.
        psum_t = tc.alloc_tile_pool(name="psum_a", bufs=2, space="PSUM")

        vecs_sb = const.tile([H, 4], F32)
        nc.gpsimd.dma_start(vecs_sb, vecs.ap())
        WkT = const.tile([128, 8, 128], F16)
        nc.scalar.dma_start(WkT, WkTd.ap().rearrange("(o p) h -> p o h", p=128))
        WvT = const.tile([128, 8, 128], F16)
        nc.scalar.dma_start(WvT, WvTd.ap().rearrange("(o p) h -> p o h", p=128))
        WqT = const.tile([128, 8, 128], F16)
        nc.scalar.dma_start(WqT, WqTd.ap().rearrange("(o p) h -> p o h", p=128))
        WoT = const.tile([128, D], F16)  # [h, d]
        nc.scalar.dma_start(WoT, WoTd.ap())

        # ---- x.T chunk loads (plain DMA; host pre-transposed) ---------
        # All 8 chunks stay resident: the K/Q pass and the later V pass
        # both read them, and reloading 8 MB would double input traffic.
        xT3 = xT.ap().rearrange("(o p) s -> p o s", p=128)
        xt_tiles = []
        for sc in range(8):
            xt = xa_pool.tile([128, 8, 512], F16, tag="xt", bufs=8,
                              name=f"xt_{sc}")
            nc.sync.dma_start(xt, xT3[:, :, sc * 512:(sc + 1) * 512])
            xt_tiles.append(xt)

        # ---- constants -------------------------------------------------
        wav_sb = vecs_sb[:, 0:1]
        phase_sb = vecs_sb[:, 1:2]
        bq_sb = vecs_sb[:, 2:3]
        bk_sb = vecs_sb[:, 3:4]

        inv_w = const.tile([H, 1], F32)
        tw = const.tile([H, 1], F32)
        nc.vector.tensor_scalar(tw, wav_sb, 1e-8, None, mybir.AluOpType.add)
        nc.vector.reciprocal(inv_w, tw)
        cadd_q = const.tile([H, 1], F32)
        nc.vector.tensor_scalar(cadd_q, bq_sb, inv_w, phase_sb,
                                mybir.AluOpType.mult, mybir.AluOpType.add)
        cadd_k = const.tile([H, 1], F32)
        nc.vector.tensor_scalar(cadd_k, bk_sb, inv_w, phase_sb,
                                mybir.AluOpType.mult, mybir.AluOpType.add)
        inv_w2 = const.tile([H, 1], F32)
        nc.vector.tensor_scalar(inv_w2, inv_w, INV_TWO_PI, None,
                                mybir.AluOpType.mult)
        cadd_q2 = const.tile([H, 1], F32)
        nc.vector.tensor_scalar(cadd_q2, cadd_q, INV_TWO_PI, None,
                                mybir.AluOpType.mult)
        cadd_k2 = const.tile([H, 1], F32)
        nc.vector.tensor_scalar(cadd_k2, cadd_k, INV_TWO_PI, None,
                                mybir.AluOpType.mult)

        neg1 = const.tile([128, 1], F32)
        nc.vector.memset(neg1, -1.0)
        cadd_q2M = const.tile([H, 1], F32)
        nc.vector.tensor_scalar(cadd_q2M, cadd_q2, MAGIC, None,
                                mybir.AluOpType.add)
        cadd_k2M = const.tile([H, 1], F32)
        nc.vector.tensor_scalar(cadd_k2M, cadd_k2, MAGIC, None,
                                mybir.AluOpType.add)
        negM = const.tile([128, 1], F32)
        nc.vector.memset(negM, -MAGIC)

        # ---- persistent activations -----------------------------------
        # fp8 cos/sin planes: [:, 0, :] = cos, [:, 1, :] = sin.
        Fq8 = big.tile([128, 2, SQ], F8)
        Fk8 = big.tile([128, 2, S], F8)
        Vn = big.tile([128, 32, 129], F16)  # [k_part, k_tile, h | ones]
        nc.vector.memset(Vn[:, :, 128:129], 1.0)
        osb = big.tile([128, 16, 129], F32)  # raw [O | denom] per q-subtile
        recs = [big.tile([128, 1], F32, name=f"rec_{i}", tag=f"rec_{i}")
                for i in range(16)]

        # ---- phase A: x.T (DMA), projections, sin/cos, V --------------
        def theta_path(pp, cadd, cadd2M_, cos_slice, sin_slice):
            # th on DVE in parallel with the u/kk magic-round on ACT keeps
            # both engines under the projection-matmul time.
            th = tmp.tile([128, 512], F32, tag="th", bufs=4)
            nc.vector.tensor_scalar(th, pp, inv_w, cadd,
                                    mybir.AluOpType.mult, mybir.AluOpType.add)
            u = tmp.tile([128, 512], F32, tag="u", bufs=4)
            nc.scalar.activation(u, pp, AF.Identity, bias=cadd2M_,
                                 scale=inv_w2)
            kk = tmp.tile([128, 512], F32, tag="kk", bufs=4)
            nc.scalar.activation(kk, u, AF.Identity, bias=negM,
                                 scale=1.0)
            thr = tmp.tile([128, 512], F32, tag="thr", bufs=4)
            nc.vector.cody_waite_cascade(thr, th, kk, C1, C2, C3)
            nc.scalar.activation(sin_slice, thr, AF.Sin)
            thc = tmp.tile([128, 512], F32, tag="thc", bufs=4)
            nc.vector.add_range_wrap(thc, thr, math.pi / 2, math.pi, TWO_PI)
            nc.scalar.activation(cos_slice, thc, AF.Sin)

        def proj(wt, xt):
            pp = psum_t.tile([128, 512], F32, tag="proj", bufs=6)
            for dc in range(8):
                nc.tensor.matmul(pp, wt[:, dc, :], xt[:, dc, :],
                                 start=(dc == 0), stop=(dc == 7))
            return pp

        # K and Q passes first: all ACT Sin work completes before phase B's
        # Exp stream, so the ACT table switches exactly once.
        for sc in range(8):
            sl = slice(sc * 512, (sc + 1) * 512)
            theta_path(proj(WkT, xt_tiles[sc]), cadd_k, cadd_k2M,
                       Fk8[:, 0, sl], Fk8[:, 1, sl])
            if sc < 4:
                theta_path(proj(WqT, xt_tiles[sc]), cadd_q, cadd_q2M,
                           Fq8[:, 0, sl], Fq8[:, 1, sl])

        # V pass last; the [s,h]-layout transpose runs on the DMA XBAR
        # (out[p,t,h] = v16[h, t*128+p]), so PSUM holds only the proj pool
        # and phase B can take over the banks right after the last proj.
        for sc in range(8):
            ppv = proj(WvT, xt_tiles[sc])
            # Deep buffering decouples the proj matmul stream from the
            # XBAR-transpose latency chain (v16 reuse would stall PE).
            v16 = tmp.tile([128, 512], F16, tag="v16", bufs=8)
            nc.scalar.copy(v16, ppv)
            # The XBAR write path is pitch-contiguous, so land in a staging
            # tile and let DVE fan it into Vn's 129-element pitch.
            vtmp = tmp.tile([128, 4, 128], F16, tag="vtmp", bufs=4)
            eng = nc.sync if sc % 2 == 0 else nc.scalar
            eng.dma_start_transpose(vtmp, v16)
            nc.vector.tensor_copy(Vn[:, sc * 4:(sc + 1) * 4, 0:128], vtmp)

        psum_t.release()

        # ---- phase B: attention per 512-query chunk -------------------
        # 11 PSUM tiles of up to 3 k-tiles each (3 banks, double-buffered)
        # cover the 32 k-tiles; one exp per tile.
        psum_b = tc.alloc_tile_pool(name="psum_b", bufs=1, space="PSUM")
        for qc in range(4):
            qsl = slice(qc * 512, (qc + 1) * 512)
            opsA = psum_b.tile([128, 3, 132], F32, tag="opsA",
                               name=f"opsA_{qc}")
            opsB = psum_b.tile([128, 129], F32, tag="opsB",
                               name=f"opsB_{qc}")
            ops = [opsA[:, 0, 0:129], opsA[:, 1, 0:129], opsA[:, 2, 0:129],
                   opsB]
            for ti in range(11):
                n_k = 3 if ti < 10 else 2
                st = psum_b.tile([128, 1536], F32, tag="st", bufs=2)
                for j in range(n_k):
                    kt = ti * 3 + j
                    nc.tensor.matmul(
                        st[:, j * 512:(j + 1) * 512],
                        Fk8[:, :, kt * 128:(kt + 1) * 128],
                        Fq8[:, :, qsl],
                        start=True, stop=True, perf_mode=DR)
                et = tmp.tile([128, 1536], F16, tag="et", bufs=2)
                nc.scalar.activation(et[:, 0:n_k * 512], st[:, 0:n_k * 512],
                                     AF.Exp, bias=neg1, scale=INV_SQRT_H)
                for j in range(n_k):
                    kt = ti * 3 + j
                    for qs in range(4):
                        # start=True zeroes the whole 2KB PSUM bank, so only
                        # the first write into opsA's bank may carry it.
                        nc.tensor.matmul(
                            ops[qs],
                            et[:, j * 512 + qs * 128:j * 512 + (qs + 1) * 128],
                            Vn[:, kt, :],
                            start=(kt == 0 and (qs == 0 or qs == 3)),
                            stop=(kt == 31),
                            skip_group_check=True)
            nc.vector.tensor_copy(osb[:, qc * 4:qc * 4 + 3, :],
                                  opsA[:, :, 0:129])
            nc.vector.tensor_copy(osb[:, qc * 4 + 3, :], opsB)
            for qs in range(4):
                i = qc * 4 + qs
                nc.vector.reciprocal(recs[i], osb[:, i, 128:129])

        psum_b.release()

        # ---- phase C: normalize + output projection -------------------
        # O.T comes from the DMA XBAR (not the PE), so PSUM holds only the
        # output-projection tiles; y rows store as single 2 KB descriptors.
        psum_c = tc.alloc_tile_pool(name="psum_c", bufs=1, space="PSUM")
        for qc in range(4):
            for qs in range(4):
                i = qc * 4 + qs
                onrm = tmp.tile([128, 128], F16, tag="onrm", bufs=4)
                nc.vector.tensor_scalar(onrm, osb[:, i, 0:128], recs[i], None,
                                        mybir.AluOpType.mult)
                ot = tmp.tile([128, 128], F16, tag="ot", bufs=4)
                eng = nc.sync if i % 2 == 0 else nc.scalar
                eng.dma_start_transpose(ot, onrm)
                ysb = tmp.tile([128, 1024], F16, tag="ysb", bufs=4)
                for half in range(2):
                    yp = psum_c.tile([128, 512], F32, tag="yp", bufs=4)
                    nc.tensor.matmul(yp, ot,
                                     WoT[:, half * 512:(half + 1) * 512],
                                     start=True, stop=True)
                    # bo is added on the host; eviction alternates DVE/ACT.
                    if (qs + half) % 2 == 0:
                        nc.vector.tensor_copy(
                            ysb[:, half * 512:(half + 1) * 512], yp)
                    else:
                        nc.scalar.copy(
                            ysb[:, half * 512:(half + 1) * 512], yp)
                row = i * 128
                eng = nc.scalar if i % 2 == 0 else nc.sync
                eng.dma_start(y.ap()[row:row + 128, :], ysb)
        psum_c.release()

    nc.compile()
    return nc


def get_nc():
    global _CACHED
    if _CACHED is None:
        _CACHED = _build()
    return _CACHED


def _in_maps(inputs):
    x = np.asarray(inputs["x"], np.float32)
    small = {
        "WqT": np.ascontiguousarray(np.asarray(inputs["Wq"], np.float16).T),
        "WkT": np.ascontiguousarray(np.asarray(inputs["Wk"], np.float16).T),
        "WvT": np.ascontiguousarray(np.asarray(inputs["Wv"], np.float16).T),
        "WoT": np.ascontiguousarray(np.asarray(inputs["Wo"], np.float16).T),
        "vecs": np.stack([
            np.asarray(inputs["wavelengths"], np.float32),
            np.asarray(inputs["phase_bias"], np.float32),
            np.asarray(inputs["bq"], np.float32),
            np.asarray(inputs["bk"], np.float32),
        ], axis=1),
    }
    maps = []
    for c in range(N_CORES):
        b, qoff = c // 2, (c % 2) * SQ
        xc = np.roll(x[b], -qoff, axis=0) if qoff else x[b]
        maps.append({"xT": np.ascontiguousarray(xc.astype(np.float16).T),
                     **small})
    return maps


def _bo_eff(inputs):
    # bv is folded through the attention average into the output bias:
    # softmax weights sum to 1, so out = softmax(..) @ (V + bv) @ Wo.T + bo
    #                                  = softmax(..) @ V @ Wo.T + (Wo @ bv + bo)
    Wo = np.asarray(inputs["Wo"], np.float32)
    bv = np.asarray(inputs["bv"], np.float32)
    return np.asarray(inputs["bo"], np.float32) + Wo @ bv


def kernel(**inputs):
    from concourse.bass_utils import run_bass_kernel_spmd

    nc = get_nc()
    res = run_bass_kernel_spmd(nc, _in_maps(inputs),
                               core_ids=list(range(N_CORES)))
    out = np.empty((B, S, D), np.float32)
    for c in range(N_CORES):
        b, qoff = c // 2, (c % 2) * SQ
        out[b, qoff:qoff + SQ] = res.results[c]["y"]
    out += _bo_eff(inputs)
    return out


# revision 26
# speedup vs baseline: 1.1449x; 1.1449x over previous
"""EulerAttentionHead Trainium2 kernel (8 NeuronCores, SPMD).

Reference computation (B=4, S=4096, D=1024, H=128):
    Q = x @ Wq.T + bq ; K = x @ Wk.T + bk ; V = x @ Wv.T + bv
    theta_{q,k} = {Q,K} / (wavelengths + 1e-8) + phase_bias
    sim = cos(tq) @ cos(tk).T + sin(tq) @ sin(tk).T
    out = softmax(sim / sqrt(H)) @ V @ Wo.T + bo

Sharding: 8 cores = 4 batches x 2 query-halves. Each core handles one
batch's full key/value set (4096 keys) and 2048 queries. The host rolls
x so each core's query rows are rows 0:2048 of its input (softmax over
keys is permutation-invariant, so key order doesn't matter).

Host prep: x and the four weight matrices are cast to fp16 and
pre-transposed in numpy so every device-side matmul operand has its
contraction dim on SBUF partitions. bv is folded into bo on the host
(softmax weights sum to 1, so + bv commutes through the attention
average into the output projection: bo' = bo + Wo @ bv).

Per-core dataflow:
  phase A: per 512-row chunk, Q.T/K.T/V.T = W.T-stationary fp16 matmuls
    over x.T; theta built on DVE (per-partition scale/bias, fp32 magic-
    number round, Cody-Waite cascade + add_range_wrap into the ACT Sin
    domain); sin/cos emitted as fp8e4 planes of [128, 2, n] tiles
    (plane 0 = cos, plane 1 = sin) for DoubleRow score matmuls. V is
    re-transposed to natural [k, h] layout on the PE with a ones column
    appended.
  phase B: per 512-query chunk, S.T k-tiles [k,128 x q,512] computed as
    ONE fp8 DoubleRow matmul each (contracts cos&sin planes, 2x FLOP
    rate); 3 k-tiles pooled per 3-bank PSUM tile so one ACT pass
    computes exp(S/sqrt(H) - 1) -> E.T fp16 over 1536 cols (amortizes
    the 352-cycle ACT fixed cost). AV: lhsT = E.T, rhs = [V | ones], so
    the softmax denominator accumulates as PSUM column 128 for free.
    Raw [O | denom] is evicted to SBUF; reciprocals run on DVE.
  phase C: normalize O rows on DVE, PE-transpose O, project with Wo.T,
    bias added on DVE during eviction, store on alternating HWDGE
    queues.

PSUM per phase: A: proj + V transpose; B: 2x 3-bank S.T tiles + 2 banks
of packed O accumulators (start=True zeroes a whole 2KB bank, so the
packed accumulators carry exactly one start per bank); C: O transpose +
output tiles.
"""

import math

import numpy as np

import concourse.mybir as mybir
import concourse.tile as tile
from concourse import bacc
from concourse.masks import make_identity

F32 = mybir.dt.float32
F16 = mybir.dt.float16
F8 = mybir.dt.float8e4
AF = mybir.ActivationFunctionType
DR = mybir.MatmulPerfMode.DoubleRow

B, S, D, H = 4, 4096, 1024, 128
SQ = S // 2  # queries per core
N_CORES = 8

TWO_PI = 2.0 * math.pi
INV_TWO_PI = 1.0 / TWO_PI
MAGIC = 12582912.0  # 1.5 * 2**23: fp32 (u + M) - M == round(u)
INV_SQRT_H = 1.0 / math.sqrt(H)


def _cody_waite_consts():
    # Split 2*pi into c1 + c2 + c3, c1/c2 with zeroed low mantissa bits so
    # theta - k*c1 - k*c2 - k*c3 cancels exactly for small integer k.
    def chop(v):
        f = np.float32(v)
        i = f.view(np.uint32) & np.uint32(0xFFFFF000)
        return float(i.view(np.float32))

    c1 = chop(TWO_PI)
    c2 = chop(TWO_PI - c1)
    c3 = float(np.float32(TWO_PI - c1 - c2))
    return c1, c2, c3


C1, C2, C3 = _cody_waite_consts()

_CACHED = None


def _build():
    nc = bacc.Bacc("TRN2", target_bir_lowering=False, debug=False,
                   num_devices=N_CORES)

    xT = nc.dram_tensor("xT", (D, S), F16, kind="ExternalInput")
    WqTd = nc.dram_tensor("WqT", (D, H), F16, kind="ExternalInput")
    WkTd = nc.dram_tensor("WkT", (D, H), F16, kind="ExternalInput")
    WvTd = nc.dram_tensor("WvT", (D, H), F16, kind="ExternalInput")
    WoTd = nc.dram_tensor("WoT", (H, D), F16, kind="ExternalInput")
    vecs = nc.dram_tensor("vecs", (H, 4), F32, kind="ExternalInput")
    # y is stored fp16 (half the store bandwidth); host upcasts + adds bo.
    y = nc.dram_tensor("y", (SQ, D), F16, kind="ExternalOutput")

    with tile.TileContext(nc) as tc, \
            tc.tile_pool(name="const", bufs=1) as const, \
            tc.tile_pool(name="big", bufs=1) as big, \
            tc.tile_pool(name="xa", bufs=2) as xa_pool, \
            tc.tile_pool(name="tmp", bufs=3) as tmp:

        # ---- input DMAs, spread across engine trigger queues so the
        # critical-path loads (WkT, xt0) are triggered by sequencers that
        # go idle earliest and don't queue behind 8 MB of x descriptors.
        psum_t = tc.alloc_tile_pool(name="psum_a", bufs=2, space="PSUM")

        vecs_sb = const.tile([H, 4], F32)
        nc.gpsimd.dma_start(vecs_sb, vecs.ap())
        WkT = const.tile([128, 8, 128], F16)
        nc.scalar.dma_start(WkT, WkTd.ap().rearrange("(o p) h -> p o h", p=128))
        WvT = const.tile([128, 8, 128], F16)
        nc.scalar.dma_start(WvT, WvTd.ap().rearrange("(o p) h -> p o h", p=128))
        WqT = const.tile([128, 8, 128], F16)
        nc.scalar.dma_start(WqT, WqTd.ap().rearrange("(o p) h -> p o h", p=128))
        WoT = const.tile([128, D], F16)  # [h, d]
        nc.scalar.dma_start(WoT, WoTd.ap())

        # ---- x.T chunk loads (plain DMA; host pre-transposed) ---------
        # All 8 chunks stay resident: the K/Q pass and the later V pass
        # both read them, and reloading 8 MB would double input traffic.
        xT3 = xT.ap().rearrange("(o p) s -> p o s", p=128)
        xt_tiles = []
        for sc in range(8):
            xt = xa_pool.tile([128, 8, 512], F16, tag="xt", bufs=8,
                              name=f"xt_{sc}")
            nc.sync.dma_start(xt, xT3[:, :, sc * 512:(sc + 1) * 512])
            xt_tiles.append(xt)

        # ---- constants -------------------------------------------------
        ident_h = const.tile([128, 128], F16)
        make_identity(nc, ident_h)

        wav_sb = vecs_sb[:, 0:1]
        phase_sb = vecs_sb[:, 1:2]
        bq_sb = vecs_sb[:, 2:3]
        bk_sb = vecs_sb[:, 3:4]

        inv_w = const.tile([H, 1], F32)
        tw = const.tile([H, 1], F32)
        nc.vector.tensor_scalar(tw, wav_sb, 1e-8, None, mybir.AluOpType.add)
        nc.vector.reciprocal(inv_w, tw)
        cadd_q = const.tile([H, 1], F32)
        nc.vector.tensor_scalar(cadd_q, bq_sb, inv_w, phase_sb,
                                mybir.AluOpType.mult, mybir.AluOpType.add)
        cadd_k = const.tile([H, 1], F32)
        nc.vector.tensor_scalar(cadd_k, bk_sb, inv_w, phase_sb,
                                mybir.AluOpType.mult, mybir.AluOpType.add)
        inv_w2 = const.tile([H, 1], F32)
        nc.vector.tensor_scalar(inv_w2, inv_w, INV_TWO_PI, None,
                                mybir.AluOpType.mult)
        cadd_q2 = const.tile([H, 1], F32)
        nc.vector.tensor_scalar(cadd_q2, cadd_q, INV_TWO_PI, None,
                                mybir.AluOpType.mult)
        cadd_k2 = const.tile([H, 1], F32)
        nc.vector.tensor_scalar(cadd_k2, cadd_k, INV_TWO_PI, None,
                                mybir.AluOpType.mult)

        neg1 = const.tile([128, 1], F32)
        nc.vector.memset(neg1, -1.0)
        cadd_q2M = const.tile([H, 1], F32)
        nc.vector.tensor_scalar(cadd_q2M, cadd_q2, MAGIC, None,
                                mybir.AluOpType.add)
        cadd_k2M = const.tile([H, 1], F32)
        nc.vector.tensor_scalar(cadd_k2M, cadd_k2, MAGIC, None,
                                mybir.AluOpType.add)
        negM = const.tile([128, 1], F32)
        nc.vector.memset(negM, -MAGIC)

        # ---- persistent activations -----------------------------------
        # fp8 cos/sin planes: [:, 0, :] = cos, [:, 1, :] = sin.
        Fq8 = big.tile([128, 2, SQ], F8)
        Fk8 = big.tile([128, 2, S], F8)
        Vn = big.tile([128, 32, 129], F16)  # [k_part, k_tile, h | ones]
        nc.vector.memset(Vn[:, :, 128:129], 1.0)
        osb = big.tile([128, 16, 129], F32)  # raw [O | denom] per q-subtile
        recs = [big.tile([128, 1], F32, name=f"rec_{i}", tag=f"rec_{i}")
                for i in range(16)]

        # ---- phase A: x.T (DMA), projections, sin/cos, V --------------
        def theta_path(pp, cadd, cadd2M_, cos_slice, sin_slice):
            # th on DVE in parallel with the u/kk magic-round on ACT keeps
            # both engines under the projection-matmul time.
            th = tmp.tile([128, 512], F32, tag="th", bufs=4)
            nc.vector.tensor_scalar(th, pp, inv_w, cadd,
                                    mybir.AluOpType.mult, mybir.AluOpType.add)
            u = tmp.tile([128, 512], F32, tag="u", bufs=4)
            nc.scalar.activation(u, pp, AF.Identity, bias=cadd2M_,
                                 scale=inv_w2)
            kk = tmp.tile([128, 512], F32, tag="kk", bufs=4)
            nc.scalar.activation(kk, u, AF.Identity, bias=negM,
                                 scale=1.0)
            thr = tmp.tile([128, 512], F32, tag="thr", bufs=4)
            nc.vector.cody_waite_cascade(thr, th, kk, C1, C2, C3)
            nc.scalar.activation(sin_slice, thr, AF.Sin)
            thc = tmp.tile([128, 512], F32, tag="thc", bufs=4)
            nc.vector.add_range_wrap(thc, thr, math.pi / 2, math.pi, TWO_PI)
            nc.scalar.activation(cos_slice, thc, AF.Sin)

        def proj(wt, xt):
            pp = psum_t.tile([128, 512], F32, tag="proj", bufs=6)
            for dc in range(8):
                nc.tensor.matmul(pp, wt[:, dc, :], xt[:, dc, :],
                                 start=(dc == 0), stop=(dc == 7))
            return pp

        for sc in range(8):
            xt = xt_tiles[sc]
            sl = slice(sc * 512, (sc + 1) * 512)
            theta_path(proj(WkT, xt), cadd_k, cadd_k2M,
                       Fk8[:, 0, sl], Fk8[:, 1, sl])

            ppv = proj(WvT, xt)
            v16 = tmp.tile([128, 512], F16, tag="v16")
            nc.scalar.copy(v16, ppv)

            if sc < 4:
                theta_path(proj(WqT, xt), cadd_q, cadd_q2M,
                           Fq8[:, 0, sl], Fq8[:, 1, sl])

            pv = psum_t.tile([128, 512], F16, tag="pt")
            for a in range(4):
                nc.tensor.transpose(pv[:, a * 128:(a + 1) * 128],
                                    v16[:, a * 128:(a + 1) * 128], ident_h)
            nc.vector.tensor_copy(
                Vn[:, sc * 4:(sc + 1) * 4, 0:128],
                pv.rearrange("p (a h) -> p a h", a=4))

        psum_t.release()

        # ---- phase B: attention per 512-query chunk -------------------
        # 11 PSUM tiles of up to 3 k-tiles each (3 banks, double-buffered)
        # cover the 32 k-tiles; one exp per tile.
        psum_b = tc.alloc_tile_pool(name="psum_b", bufs=1, space="PSUM")
        for qc in range(4):
            qsl = slice(qc * 512, (qc + 1) * 512)
            opsA = psum_b.tile([128, 3, 132], F32, tag="opsA",
                               name=f"opsA_{qc}")
            opsB = psum_b.tile([128, 129], F32, tag="opsB",
                               name=f"opsB_{qc}")
            ops = [opsA[:, 0, 0:129], opsA[:, 1, 0:129], opsA[:, 2, 0:129],
                   opsB]
            for ti in range(11):
                n_k = 3 if ti < 10 else 2
                st = psum_b.tile([128, 1536], F32, tag="st", bufs=2)
                for j in range(n_k):
                    kt = ti * 3 + j
                    nc.tensor.matmul(
                        st[:, j * 512:(j + 1) * 512],
                        Fk8[:, :, kt * 128:(kt + 1) * 128],
                        Fq8[:, :, qsl],
                        start=True, stop=True, perf_mode=DR)
                et = tmp.tile([128, 1536], F16, tag="et", bufs=2)
                nc.scalar.activation(et[:, 0:n_k * 512], st[:, 0:n_k * 512],
                                     AF.Exp, bias=neg1, scale=INV_SQRT_H)
                for j in range(n_k):
                    kt = ti * 3 + j
                    for qs in range(4):
                        # start=True zeroes the whole 2KB PSUM bank, so only
                        # the first write into opsA's bank may carry it.
                        nc.tensor.matmul(
                            ops[qs],
                            et[:, j * 512 + qs * 128:j * 512 + (qs + 1) * 128],
                            Vn[:, kt, :],
                            start=(kt == 0 and (qs == 0 or qs == 3)),
                            stop=(kt == 31),
                            skip_group_check=True)
            nc.vector.tensor_copy(osb[:, qc * 4:qc * 4 + 3, :],
                                  opsA[:, :, 0:129])
            nc.vector.tensor_copy(osb[:, qc * 4 + 3, :], opsB)
            for qs in range(4):
                i = qc * 4 + qs
                nc.vector.reciprocal(recs[i], osb[:, i, 128:129])

        psum_b.release()

        # ---- phase C: normalize + output projection -------------------
        # O.T comes from the DMA XBAR (not the PE), so PSUM holds only the
        # output-projection tiles; y rows store as single 2 KB descriptors.
        psum_c = tc.alloc_tile_pool(name="psum_c", bufs=1, space="PSUM")
        for qc in range(4):
            for qs in range(4):
                i = qc * 4 + qs
                onrm = tmp.tile([128, 128], F16, tag="onrm", bufs=4)
                nc.vector.tensor_scalar(onrm, osb[:, i, 0:128], recs[i], None,
                                        mybir.AluOpType.mult)
                otp = psum_c.tile([128, 128], F16, tag="ptc", bufs=4)
                nc.tensor.transpose(otp, onrm, ident_h)
                ot = tmp.tile([128, 128], F16, tag="ot", bufs=4)
                nc.vector.tensor_copy(ot, otp)
                ysb = tmp.tile([128, 1024], F16, tag="ysb", bufs=4)
                for half in range(2):
                    yp = psum_c.tile([128, 512], F32, tag="yp", bufs=4)
                    nc.tensor.matmul(yp, ot,
                                     WoT[:, half * 512:(half + 1) * 512],
                                     start=True, stop=True)
                    # bo is added on the host; eviction alternates DVE/ACT.
                    if (qs + half) % 2 == 0:
                        nc.vector.tensor_copy(
                            ysb[:, half * 512:(half + 1) * 512], yp)
                    else:
                        nc.scalar.copy(
                            ysb[:, half * 512:(half + 1) * 512], yp)
                row = i * 128
                eng = nc.scalar if i % 2 == 0 else nc.sync
                eng.dma_start(y.ap()[row:row + 128, :], ysb)
        psum_c.release()

    nc.compile()
    return nc


def get_nc():
    global _CACHED
    if _CACHED is None:
        _CACHED = _build()
    return _CACHED


def _in_maps(inputs):
    x = np.asarray(inputs["x"], np.float32)
    small = {
        "WqT": np.ascontiguousarray(np.asarray(inputs["Wq"], np.float16).T),
        "WkT": np.ascontiguousarray(np.asarray(inputs["Wk"], np.float16).T),
        "WvT": np.ascontiguousarray(np.asarray(inputs["Wv"], np.float16).T),
        "WoT": np.ascontiguousarray(np.asarray(inputs["Wo"], np.float16).T),
        "vecs": np.stack([
            np.asarray(inputs["wavelengths"], np.float32),
            np.asarray(inputs["phase_bias"], np.float32),
            np.asarray(inputs["bq"], np.float32),
            np.asarray(inputs["bk"], np.float32),
        ], axis=1),
    }
    maps = []
    for c in range(N_CORES):
        b, qoff = c // 2, (c % 2) * SQ
        xc = np.roll(x[b], -qoff, axis=0) if qoff else x[b]
        maps.append({"xT": np.ascontiguousarray(xc.astype(np.float16).T),
                     **small})
    return maps


def _bo_eff(inputs):
    # bv is folded through the attention average into the output bias:
    # softmax weights sum to 1, so out = softmax(..) @ (V + bv) @ Wo.T + bo
    #                                  = softmax(..) @ V @ Wo.T + (Wo @ bv + bo)
    Wo = np.asarray(inputs["Wo"], np.float32)
    bv = np.asarray(inputs["bv"], np.float32)
    return np.asarray(inputs["bo"], np.float32) + Wo @ bv


def kernel(**inputs):
    from concourse.bass_utils import run_bass_kernel_spmd

    nc = get_nc()
    res = run_bass_kernel_spmd(nc, _in_maps(inputs),
                               core_ids=list(range(N_CORES)))
    out = np.empty((B, S, D), np.float32)
    for c in range(N_CORES):
        b, qoff = c // 2, (c % 2) * SQ
        out[b, qoff:qoff + SQ] = res.results[c]["y"]
    out += _bo_eff(inputs)
    return out
